# revision 1
# baseline (speedup 1.0000x reference)
"""Multihead attention (B=2, S=2048, D=1024, H=16) on 8 TRN2 NeuronCores.

Sharding: core c -> batch b = c//4, head-group g = c%4 (4 heads, 256 features).
Each core computes q/k/v projections for its 256 features, attention for its
4 heads, and a row-parallel partial of the output projection. Host sums the
4 partials per batch (row-parallel TP unshard) and transposes back.

Per-core pipeline (all matmuls bf16 with f32 PSUM accumulation):
  qT,kT [256,2048] feature-major projections (k-outer, consume input DMA as it
  streams); va [2048, 4*65] v in [s,f] layout with a ones column per head.
  Attention per head, per j-tile: scoresT [128 j, 1024 i] x2 i-halves in PSUM,
  exp(scale*x + mask_j) fused on ScalarE (mask = per-partition bias), then the
  AV matmul with va stationary: po[65, i] += va_j^T @ expT_j. The ones column
  makes po row 64 the softmax denominator. expT tiles die within their own
  j-step, so there are no cross-phase buffer stalls. The divide is a DVE
  reciprocal of row 64, a GpSimd partition-broadcast, a DVE multiply, and an
  SBUF->SBUF DMA into the [f, i]-layout OT tile (handles odd heads' partition
  offset). Output projection reads OT directly (no PE transposes anywhere).
"""

import math

import numpy as np

B, S, D, H = 2, 2048, 1024, 16
NCORES = 8
GH = 4                  # heads per core
HD = D // H             # 64
F = GH * HD             # 256 local features
SCALE = 1.0 / math.sqrt(HD)
NEG = np.float32(-9e9)

KT = D // 128           # 8 contraction tiles (projections)
FT = F // 128           # 2 local-feature tiles
ST = S // 128           # 16 sequence tiles
DT = D // 128           # 8 output-feature tiles

TRACE = False           # set by test harness; requires antenv.axon_hooks wired
LAST_EXEC_NS = None
LAST_RESULTS = None

_STATE = {}


def _build():
    import concourse.bacc as bacc
    import concourse.mybir as mybir
    from concourse.tile import TileContext

    f32 = mybir.dt.float32
    bf16 = mybir.dt.bfloat16
    Exp = mybir.ActivationFunctionType.Exp

    nc = bacc.Bacc("TRN2", target_bir_lowering=False, debug=False,
                   num_devices=NCORES)

    xq_d = nc.declare_dram_parameter("xqT", [D, S], bf16, isOutput=False)
    xk_d = nc.declare_dram_parameter("xkT", [D, S], bf16, isOutput=False)
    # xv is host-pre-tiled st-major: xv3[st, p, k*128+c] = value.T[k*128+p, st*128+c]
    xv_d = nc.declare_dram_parameter("xv3", [ST, 128, D], bf16, isOutput=False)
    wq_d = nc.declare_dram_parameter("wqT", [D, F], bf16, isOutput=False)
    wk_d = nc.declare_dram_parameter("wkT", [D, F], bf16, isOutput=False)
    wv_d = nc.declare_dram_parameter("wvT", [D, F], bf16, isOutput=False)
    wo_d = nc.declare_dram_parameter("woT", [F, D], bf16, isOutput=False)
    # partition-major pre-tiled constants: col j holds elements [j*128, (j+1)*128)
    bq_d = nc.declare_dram_parameter("bq2", [128, FT], f32, isOutput=False)
    bk_d = nc.declare_dram_parameter("bk2", [128, FT], f32, isOutput=False)
    bv_d = nc.declare_dram_parameter("bv", [F], bf16, isOutput=False)
    bo_d = nc.declare_dram_parameter("bo2", [128, DT], f32, isOutput=False)
    mk_d = nc.declare_dram_parameter("mask2", [128, ST], f32, isOutput=False)
    out_d = nc.declare_dram_parameter("outT", [D, S], bf16, isOutput=True)

    with TileContext(nc) as tc:
        with tc.tile_pool(name="persist", bufs=1) as pp, \
             tc.tile_pool(name="xin", bufs=16) as xp, \
             tc.tile_pool(name="expp", bufs=6) as ep, \
             tc.tile_pool(name="ostage", bufs=4) as osp, \
             tc.tile_pool(name="divp", bufs=2) as dp, \
             tc.tile_pool(name="small", bufs=4) as sp:

            def ptile(shape, dtype, name):
                return pp.tile(shape, dtype, name=name, tag=name)

            # ---- persistent SBUF tensors ----
            wq_sb = [ptile([128, F], bf16, f"wq{k}") for k in range(KT)]
            wk_sb = [ptile([128, F], bf16, f"wk{k}") for k in range(KT)]
            wv_sb = [ptile([128, F], bf16, f"wv{k}") for k in range(KT)]
            wo_sb = [ptile([128, D], bf16, f"wo{t}") for t in range(FT)]
            bqt = ptile([128, FT], f32, "bqt")
            bkt = ptile([128, FT], f32, "bkt")
            bot = ptile([128, DT], f32, "bot")
            mkt = ptile([128, ST], f32, "mkt")
            bq_sb = [bqt[:, t:t + 1] for t in range(FT)]
            bk_sb = [bkt[:, t:t + 1] for t in range(FT)]
            bo_sb = [bot[:, t:t + 1] for t in range(DT)]
            mk_sb = [mkt[:, j:j + 1] for j in range(ST)]
            bv_sb = ptile([1, F], bf16, "bvrow")
            ones_sb = ptile([1, 128], bf16, "onesrow")
            qT_sb = [ptile([128, S], bf16, f"qT{t}") for t in range(FT)]
            kT_sb = [ptile([128, S], bf16, f"kT{t}") for t in range(FT)]
            va_sb = [ptile([128, GH * 128], bf16, f"va{j}") for j in range(ST)]
            ot_sb = [ptile([128, S], bf16, f"ot{t}") for t in range(FT)]

            nc.vector.memset(ones_sb[:], 1.0)
            # va per-head block of 128 cols: [ones | zeros(63) | v(64)] so the
            # AV output has the denominator in row 0 and features at the
            # 32-aligned partition offset 64
            for j in range(ST):
                nc.vector.memset(va_sb[j][:], 0.0)
                for h in range(GH):
                    nc.vector.memset(va_sb[j][:, h * 128:h * 128 + 1], 1.0)

            # DMAs in consumption order: tiny constants, K stream, Q stream,
            # V stream, then the output-projection weights.
            nc.sync.dma_start(out=mkt[:], in_=mk_d[:])
            nc.sync.dma_start(out=bqt[:], in_=bq_d[:])
            nc.sync.dma_start(out=bkt[:], in_=bk_d[:])
            nc.sync.dma_start(out=bot[:], in_=bo_d[:])
            nc.sync.dma_start(out=bv_sb[:], in_=bv_d[:].unsqueeze(0))

            xq_sb, xk_sb, xv_sb = [], [], []
            for w_sb, w_d, x_sb, x_d, nm in ((wk_sb, wk_d, xk_sb, xk_d, "xk"),
                                             (wq_sb, wq_d, xq_sb, xq_d, "xq")):
                for k in range(KT):
                    nc.sync.dma_start(out=w_sb[k][:],
                                      in_=w_d[k * 128:(k + 1) * 128, :])
                    xt = xp.tile([128, S], bf16, name=f"{nm}{k}", tag="xin")
                    nc.sync.dma_start(out=xt[:], in_=x_d[k * 128:(k + 1) * 128, :])
                    x_sb.append(xt)
            for k in range(KT):
                nc.sync.dma_start(out=wv_sb[k][:], in_=wv_d[k * 128:(k + 1) * 128, :])
            for st in range(ST):
                xt = xp.tile([128, D], bf16, name=f"xv{st}", tag="xvp", bufs=4)
                nc.sync.dma_start(out=xt[:], in_=xv_d[st])
                xv_sb.append(xt)
            for t in range(FT):
                nc.sync.dma_start(out=wo_sb[t][:], in_=wo_d[t * 128:(t + 1) * 128, :])

            with tc.tile_pool(name="psB", bufs=2, space="PSUM") as psB:

                def ps_tile(name, tag):
                    return psB.tile([128, 1024], mybir.dt.float32,
                                    name=name, tag=tag)

                # q/k projection: 4 (f-tile, s-half) accumulators across both
                # PSUM tags, k-outer so each streamed input tile is consumed
                # (and its xin slot freed) the moment its DMA lands.
                def proj_qk(w_sb, x_sb, b_sb, y_sb):
                    accs = {}
                    for i, (t, sh) in enumerate([(0, 0), (0, 1), (1, 0), (1, 1)]):
                        accs[t, sh] = ps_tile("acc", "pssc" if i < 2 else "pav")
                    for k in range(KT):
                        for (t, sh), acc in accs.items():
                            s0 = sh * 1024
                            for n in range(2):
                                nc.tensor.matmul(
                                    acc[:, n * 512:(n + 1) * 512],
                                    lhsT=w_sb[k][:, t * 128:(t + 1) * 128],
                                    rhs=x_sb[k][:, s0 + n * 512:s0 + (n + 1) * 512],
                                    start=(k == 0), stop=(k == KT - 1))
                    for (t, sh), acc in accs.items():
                        nc.vector.tensor_scalar_add(
                            y_sb[t][:, sh * 1024:(sh + 1) * 1024],
                            acc[:], b_sb[t])

                # v projection for one seq tile (+bias via ones-row matmul)
                def vproj_unit(st):
                    pv = psB.tile([128, F], mybir.dt.float32,
                                  name="pv", tag="pssc")
                    for k in range(KT):
                        nc.tensor.matmul(
                            pv[:], lhsT=xv_sb[st][:, k * 128:(k + 1) * 128],
                            rhs=wv_sb[k][:], start=(k == 0), stop=False)
                    nc.tensor.matmul(pv[:], lhsT=ones_sb[:], rhs=bv_sb[:],
                                     start=False, stop=True)
                    for h in range(GH):
                        nc.vector.tensor_copy(
                            va_sb[st][:, h * 128 + HD:(h + 1) * 128],
                            pv[:, h * HD:(h + 1) * HD])

                def out_proj(ih):
                    i0 = ih * 1024
                    for do in range(DT):
                        pso = ps_tile("pso", "pssc" if do % 2 == 0 else "pav")
                        for n in range(2):
                            for t in range(FT):
                                nc.tensor.matmul(
                                    pso[:, n * 512:(n + 1) * 512],
                                    lhsT=wo_sb[t][:, do * 128:(do + 1) * 128],
                                    rhs=ot_sb[t][:, i0 + n * 512:i0 + (n + 1) * 512],
                                    start=(t == 0), stop=(t == FT - 1))
                        stg = osp.tile([128, 1024], bf16,
                                       name="stg", tag="stg")
                        if do % 2 == 0:
                            nc.vector.tensor_scalar_add(stg[:], pso[:], bo_sb[do])
                        else:
                            nc.scalar.add(stg[:], pso[:], bo_sb[do])
                        nc.sync.dma_start(
                            out=out_d[do * 128:(do + 1) * 128, i0:i0 + 512],
                            in_=stg[:, 0:512])
                        nc.sync.dma_start(
                            out=out_d[do * 128:(do + 1) * 128, i0 + 512:i0 + 1024],
                            in_=stg[:, 512:1024])

                # ---------------- emission schedule ----------------
                proj_qk(wk_sb, xk_sb, bk_sb, kT_sb)
                proj_qk(wq_sb, xq_sb, bq_sb, qT_sb)

                for h in range(GH):
                    ht = h // 2
                    off = (h % 2) * HD
                    vsl = va_sb  # per-j stationary v (+ones) slices below
                    po = [ps_tile(f"po{half}", "pav") for half in range(2)]
                    for j in range(ST):
                        if h == 0:
                            vproj_unit(j)
                        ets = []
                        for half in range(2):
                            i0 = half * 1024
                            ps = ps_tile("pssc", "pssc")
                            for n in range(2):
                                nc.tensor.matmul(
                                    ps[:, n * 512:(n + 1) * 512],
                                    lhsT=kT_sb[ht][off:off + HD,
                                                   j * 128:(j + 1) * 128],
                                    rhs=qT_sb[ht][off:off + HD,
                                                  i0 + n * 512:i0 + (n + 1) * 512],
                                    start=True, stop=True)
                            e = ep.tile([128, 1024], bf16, name="expT", tag="expT")
                            nc.scalar.activation(e[:], ps[:], Exp,
                                                 bias=mk_sb[j], scale=SCALE)
                            ets.append(e)
                        # AV with va stationary: po[65, i] += va_j^T @ expT_j
                        for half in range(2):
                            for n in range(2):
                                nc.tensor.matmul(
                                    po[half][:, n * 512:(n + 1) * 512],
                                    lhsT=vsl[j][:, h * 128:(h + 1) * 128],
                                    rhs=ets[half][:, n * 512:(n + 1) * 512],
                                    start=(j == 0), stop=(j == ST - 1))
                    # softmax divide: po row 64 is the denominator. First
                    # evacuate PSUM with one fast copy so the accumulator slot
                    # frees for the next head; the divide chain then runs off
                    # the critical path. On the last head each half's output
                    # projection follows its divide immediately.
                    for half in range(2):
                        pox = dp.tile([128, 1024], f32, name="pox", tag="pox")
                        nc.vector.tensor_copy(pox[0:1, :], po[half][0:1, :])
                        nc.vector.tensor_copy(pox[64:128, :], po[half][64:128, :])
                        rec = sp.tile([1, 1024], f32, name="rec", tag="rec")
                        nc.vector.reciprocal_approx_fast(
                            out=rec[:], in_=pox[0:1, :])
                        recb = dp.tile([128, 1024], f32, name="recb", tag="recb")
                        nc.gpsimd.partition_broadcast(recb[:], rec[:])
                        tmp = dp.tile([HD, 1024], bf16, name="tmp", tag="tmp")
                        nc.vector.tensor_tensor(
                            out=tmp[:], in0=pox[64:128, :], in1=recb[64:128, :],
                            op=mybir.AluOpType.mult)
                        nc.sync.dma_start(
                            out=ot_sb[ht][off:off + HD,
                                          half * 1024:(half + 1) * 1024],
                            in_=tmp[:])
                        if h == GH - 1:
                            warm = ps_tile("warm", "pav")
                            for wn in range(6):
                                nc.tensor.matmul(
                                    warm[:, (wn % 2) * 512:(wn % 2) * 512 + 512],
                                    lhsT=wo_sb[0][:, 0:128],
                                    rhs=qT_sb[0][:, 0:512],
                                    start=True, stop=True)
                            out_proj(half)

    nc.compile()
    return nc


def kernel(query, key, value, src_mask, Wq, bq, Wk, bk, Wv, bv, Wo, bo, nhead):
    global LAST_EXEC_NS, LAST_RESULTS
    import ml_dtypes
    from concourse.bass_utils import run_bass_kernel_spmd

    assert int(nhead) == H
    bf16 = ml_dtypes.bfloat16
    query = np.asarray(query, dtype=np.float32)
    key = np.asarray(key, dtype=np.float32)
    value = np.asarray(value, dtype=np.float32)
    src_mask = np.asarray(src_mask)
    Wq, bq = np.asarray(Wq, np.float32), np.asarray(bq, np.float32)
    Wk, bk = np.asarray(Wk, np.float32), np.asarray(bk, np.float32)
    Wv, bv = np.asarray(Wv, np.float32), np.asarray(bv, np.float32)
    Wo, bo = np.asarray(Wo, np.float32), np.asarray(bo, np.float32)

    if "nc" not in _STATE:
        _STATE["nc"] = _build()
    nc = _STATE["nc"]

    xqT = [np.ascontiguousarray(query[b].T).astype(bf16) for b in range(B)]
    xkT = [np.ascontiguousarray(key[b].T).astype(bf16) for b in range(B)]
    # st-major pre-tiling: xv3[st, p, k*128+c] = value[b].T[k*128+p, st*128+c]
    xvT = [np.ascontiguousarray(
        value[b].T.reshape(KT, 128, ST, 128).transpose(2, 1, 0, 3)
        .reshape(ST, 128, D)).astype(bf16) for b in range(B)]
    maskf = [np.ascontiguousarray(
        np.where(src_mask[b], NEG, np.float32(0)).astype(np.float32)
        .reshape(ST, 128).T) for b in range(B)]

    wqT, wkT, wvT, woT, bqs, bks, bvs = [], [], [], [], [], [], []
    for g in range(NCORES // B):
        gs, ge = g * F, (g + 1) * F
        wqT.append(np.ascontiguousarray(Wq[gs:ge, :].T).astype(bf16))
        wkT.append(np.ascontiguousarray(Wk[gs:ge, :].T).astype(bf16))
        wvT.append(np.ascontiguousarray(Wv[gs:ge, :].T).astype(bf16))
        woT.append(np.ascontiguousarray(Wo[:, gs:ge].T).astype(bf16))
        bqs.append(np.ascontiguousarray(bq[gs:ge].reshape(FT, 128).T))
        bks.append(np.ascontiguousarray(bk[gs:ge].reshape(FT, 128).T))
        bvs.append(bv[gs:ge].astype(bf16))
    bo2 = np.ascontiguousarray(bo.reshape(DT, 128).T)
    bo_zero = np.zeros_like(bo2)

    in_maps = []
    for c in range(NCORES):
        b, g = c // (NCORES // B), c % (NCORES // B)
        in_maps.append({
            "xqT": xqT[b], "xkT": xkT[b], "xv3": xvT[b],
            "wqT": wqT[g], "wkT": wkT[g], "wvT": wvT[g], "woT": woT[g],
            "bq2": bqs[g], "bk2": bks[g], "bv": bvs[g],
            "bo2": bo2 if g == 0 else bo_zero,
            "mask2": maskf[b],
        })

    kwargs = {}
    if TRACE:
        kwargs = dict(trace=True)
    res = run_bass_kernel_spmd(nc, in_maps, core_ids=list(range(NCORES)),
                               **kwargs)
    LAST_EXEC_NS = res.exec_time_ns
    LAST_RESULTS = res

    out = np.empty((B, S, D), dtype=np.float32)
    for b in range(B):
        acc = res.results[b * (NCORES // B)]["outT"].astype(np.float32)
        for g in range(1, NCORES // B):
            acc = acc + res.results[b * (NCORES // B) + g]["outT"]
        out[b] = acc.T
    return out



# revision 2
# speedup vs baseline: 1.2909x; 1.2909x over previous
"""Multihead attention (B=2, S=2048, D=1024, H=16) on 8 TRN2 NeuronCores.

Sharding: core c -> batch b = c//4, head-group g = c%4 (4 heads, 256 features).
Each core computes q/k/v projections for its 256 features, attention for its
4 heads, and a row-parallel partial of the output projection. Host sums the
4 partials per batch (row-parallel TP unshard) and transposes back.

Key compaction: src_mask keys contribute exactly 0 (exp underflow), so the
host gathers only unmasked keys/values (M_b ~ 1024 of 2048) padded to MT
tiles of 128. Pad keys keep the -9e9 bias -> exp 0. All of kproj, vproj,
scores, exp, and AV shrink by ~MT/16.

Schedule: kproj -> vproj -> qproj (PSUM banks ring-shared via 4 tags), then
attention over (i-half, head-pair) segments: per j-tile, the even head's
score matmuls run on PE row-tile T0 (SBUF partitions 0:64) and the odd
head's on T8 (64:128) concurrently (tile_position auto-derived from the
base partition), doubling score throughput. exp(scale*x + maskbias) on
ScalarE; AV accumulates [den|features] per head via the ones-column trick.
The softmax divide (DVE reciprocal, GpSimd partition-broadcast, DVE mult)
writes straight into the ot tiles. Output projection at the tail with bias
adds split across VectorE/ScalarE.
"""

import math

import numpy as np

B, S, D, H = 2, 2048, 1024, 16
NCORES = 8
GH = 4                  # heads per core
HD = D // H             # 64
F = GH * HD             # 256 local features
SCALE = 1.0 / math.sqrt(HD)
NEG = np.float32(-9e9)

KT = D // 128           # 8 contraction tiles (projections)
FT = F // 128           # 2 local-feature tiles
DT = D // 128           # 8 output-feature tiles

TRACE = False
LAST_EXEC_NS = None
LAST_RESULTS = None

_STATE = {}


def _build(MT):
    import concourse.bacc as bacc
    import concourse.mybir as mybir
    from concourse.tile import TileContext

    f32 = mybir.dt.float32
    bf16 = mybir.dt.bfloat16
    Exp = mybir.ActivationFunctionType.Exp
    MP = MT * 128

    nc = bacc.Bacc("TRN2", target_bir_lowering=False, debug=False,
                   num_devices=NCORES)

    xq_d = nc.declare_dram_parameter("xqT", [D, S], bf16, isOutput=False)
    xk_d = nc.declare_dram_parameter("xkT", [D, MP], bf16, isOutput=False)
    # xv pre-tiled st-major: xv3[st, p, k*128+c] = vc.T[k*128+p, st*128+c]
    xv_d = nc.declare_dram_parameter("xv3", [MT, 128, D], bf16, isOutput=False)
    wq_d = nc.declare_dram_parameter("wqT", [D, F], bf16, isOutput=False)
    wk_d = nc.declare_dram_parameter("wkT", [D, F], bf16, isOutput=False)
    wv_d = nc.declare_dram_parameter("wvT", [D, F], bf16, isOutput=False)
    wo_d = nc.declare_dram_parameter("woT", [F, D], bf16, isOutput=False)
    bq_d = nc.declare_dram_parameter("bq2", [128, FT], f32, isOutput=False)
    bk_d = nc.declare_dram_parameter("bk2", [128, FT], f32, isOutput=False)
    bv_d = nc.declare_dram_parameter("bvb", [128, F], f32, isOutput=False)
    bo_d = nc.declare_dram_parameter("bo2", [128, DT], f32, isOutput=False)
    mk_d = nc.declare_dram_parameter("mask2", [128, MT], f32, isOutput=False)
    out_d = nc.declare_dram_parameter("outT", [D, S], bf16, isOutput=True)

    # kproj free-dim chunks of the MP columns (PSUM bank = 512 f32)
    kchunks = []
    c0 = 0
    while c0 < MP:
        w = min(512, MP - c0)
        kchunks.append((c0, w))
        c0 += w

    with TileContext(nc) as tc:
        with tc.tile_pool(name="persist", bufs=1) as pp, \
             tc.tile_pool(name="xkp", bufs=8) as xkp, \
             tc.tile_pool(name="xvp", bufs=4) as xvp, \
             tc.tile_pool(name="xqp", bufs=8) as xqp, \
             tc.tile_pool(name="expp", bufs=6) as ep, \
             tc.tile_pool(name="divp", bufs=2) as dp, \
             tc.tile_pool(name="ostage", bufs=4) as osp, \
             tc.tile_pool(name="small", bufs=2) as sp:

            def ptile(shape, dtype, name):
                return pp.tile(shape, dtype, name=name, tag=name)

            # ---- persistent SBUF tensors ----
            wq_sb = [ptile([128, F], bf16, f"wq{k}") for k in range(KT)]
            wk_sb = [ptile([128, F], bf16, f"wk{k}") for k in range(KT)]
            wv_sb = [ptile([128, F], bf16, f"wv{k}") for k in range(KT)]
            wo_sb = [ptile([128, D], bf16, f"wo{t}") for t in range(FT)]
            bqt = ptile([128, FT], f32, "bqt")
            bkt = ptile([128, FT], f32, "bkt")
            bot = ptile([128, DT], f32, "bot")
            mkt = ptile([128, MT], f32, "mkt")
            bvb = ptile([128, F], f32, "bvb")
            qT_sb = [ptile([128, S], bf16, f"qT{t}") for t in range(FT)]
            kT_sb = [ptile([128, MP], bf16, f"kT{t}") for t in range(FT)]
            va_sb = [ptile([128, GH * 128], bf16, f"va{j}") for j in range(MT)]
            ot_sb = [ptile([128, S], bf16, f"ot{t}") for t in range(FT)]

            # va per-head block of 128 cols: [ones | zeros(63) | v(64)] so the
            # AV output has the denominator in row 0 and features at
            # partition offset 64. Pad keys are killed by the -9e9 bias.
            for j in range(MT):
                nc.vector.memset(va_sb[j][:], 0.0)
                for h in range(GH):
                    nc.vector.memset(va_sb[j][:, h * 128:h * 128 + 1], 1.0)

            # DMAs in consumption order.
            nc.sync.dma_start(out=mkt[:], in_=mk_d[:])
            nc.sync.dma_start(out=bkt[:], in_=bk_d[:])
            nc.sync.dma_start(out=bvb[:], in_=bv_d[:])
            nc.sync.dma_start(out=bqt[:], in_=bq_d[:])
            nc.sync.dma_start(out=bot[:], in_=bo_d[:])

            xk_sb, xv_sb, xq_sb = [], [], []
            for k in range(KT):
                nc.sync.dma_start(out=wk_sb[k][:],
                                  in_=wk_d[k * 128:(k + 1) * 128, :])
                xt = xkp.tile([128, MP], bf16, name=f"xk{k}", tag="xk")
                nc.sync.dma_start(out=xt[:], in_=xk_d[k * 128:(k + 1) * 128, :])
                xk_sb.append(xt)
            for k in range(KT):
                nc.sync.dma_start(out=wv_sb[k][:],
                                  in_=wv_d[k * 128:(k + 1) * 128, :])
            for st in range(MT):
                xt = xvp.tile([128, D], bf16, name=f"xv{st}", tag="xv")
                nc.sync.dma_start(out=xt[:], in_=xv_d[st])
                xv_sb.append(xt)
            for k in range(KT):
                nc.sync.dma_start(out=wq_sb[k][:],
                                  in_=wq_d[k * 128:(k + 1) * 128, :])
                xt = xqp.tile([128, S], bf16, name=f"xq{k}", tag="xq")
                nc.sync.dma_start(out=xt[:], in_=xq_d[k * 128:(k + 1) * 128, :])
                xq_sb.append(xt)
            for t in range(FT):
                nc.sync.dma_start(out=wo_sb[t][:],
                                  in_=wo_d[t * 128:(t + 1) * 128, :])

            with tc.tile_pool(name="psB", bufs=1, space="PSUM") as psB:

                TAGS = ["psA", "psB", "psC", "psD"]

                def ps_tile(tag):
                    return psB.tile([128, 1024], mybir.dt.float32,
                                    name=tag, tag=tag)

                # ---- kproj: groups (t, chunk) packed into psA..psC ----
                kg = [(t, c0, w) for t in range(FT) for (c0, w) in kchunks]
                accs = []
                for gi, (t, c0, w) in enumerate(kg):
                    if gi % 2 == 0:
                        tile = ps_tile(TAGS[gi // 2])
                    off = (gi % 2) * 512
                    accs.append(tile[:, off:off + w])
                for k in range(KT):
                    for gi, (t, c0, w) in enumerate(kg):
                        nc.tensor.matmul(
                            accs[gi],
                            lhsT=wk_sb[k][:, t * 128:(t + 1) * 128],
                            rhs=xk_sb[k][:, c0:c0 + w],
                            start=(k == 0), stop=(k == KT - 1))
                for gi, (t, c0, w) in enumerate(kg):
                    nc.vector.tensor_scalar_add(
                        kT_sb[t][:, c0:c0 + w], accs[gi], bkt[:, t:t + 1])

                # ---- vproj (ping-pong psD/psA) ----
                for st in range(MT):
                    pv = ps_tile(TAGS[3] if st % 2 == 0 else TAGS[0])
                    for k in range(KT):
                        nc.tensor.matmul(
                            pv[:, 0:F],
                            lhsT=xv_sb[st][:, k * 128:(k + 1) * 128],
                            rhs=wv_sb[k][:], start=(k == 0), stop=(k == KT - 1))
                    for h in range(GH):
                        nc.vector.tensor_tensor(
                            out=va_sb[st][:, h * 128 + HD:(h + 1) * 128],
                            in0=pv[:, h * HD:(h + 1) * HD],
                            in1=bvb[:, h * HD:(h + 1) * HD],
                            op=mybir.AluOpType.add)

                # ---- qproj: 8 groups (t, ch) across all four tags ----
                qg = [(t, ch) for t in range(FT) for ch in range(4)]
                qaccs = []
                for gi, (t, ch) in enumerate(qg):
                    if gi % 2 == 0:
                        tile = ps_tile(TAGS[gi // 2])
                    off = (gi % 2) * 512
                    qaccs.append(tile[:, off:off + 512])
                for k in range(KT):
                    for gi, (t, ch) in enumerate(qg):
                        nc.tensor.matmul(
                            qaccs[gi],
                            lhsT=wq_sb[k][:, t * 128:(t + 1) * 128],
                            rhs=xq_sb[k][:, ch * 512:(ch + 1) * 512],
                            start=(k == 0), stop=(k == KT - 1))
                for gi, (t, ch) in enumerate(qg):
                    nc.vector.tensor_scalar_add(
                        qT_sb[t][:, ch * 512:(ch + 1) * 512],
                        qaccs[gi], bqt[:, t:t + 1])

                # ---- attention ----
                for half in range(2):
                    i0 = half * 1024
                    for pair in range(FT):
                        hA, hB = 2 * pair, 2 * pair + 1
                        poA = ps_tile("psC")
                        poB = ps_tile("psD")
                        for j in range(MT):
                            sa = ps_tile("psA")
                            sb = ps_tile("psB")
                            for n in range(2):
                                for off, s in ((0, sa), (HD, sb)):
                                    nc.tensor.matmul(
                                        s[:, n * 512:(n + 1) * 512],
                                        lhsT=kT_sb[pair][off:off + HD,
                                                         j * 128:(j + 1) * 128],
                                        rhs=qT_sb[pair][off:off + HD,
                                                        i0 + n * 512:
                                                        i0 + (n + 1) * 512],
                                        start=True, stop=True)
                            eA = ep.tile([128, 1024], bf16, name="eA", tag="e")
                            nc.scalar.activation(eA[:], sa[:], Exp,
                                                 bias=mkt[:, j:j + 1],
                                                 scale=SCALE)
                            eB = ep.tile([128, 1024], bf16, name="eB", tag="e")
                            nc.scalar.activation(eB[:], sb[:], Exp,
                                                 bias=mkt[:, j:j + 1],
                                                 scale=SCALE)
                            for h, po, e in ((hA, poA, eA), (hB, poB, eB)):
                                for n in range(2):
                                    nc.tensor.matmul(
                                        po[:, n * 512:(n + 1) * 512],
                                        lhsT=va_sb[j][:, h * 128:(h + 1) * 128],
                                        rhs=e[:, n * 512:(n + 1) * 512],
                                        start=(j == 0), stop=(j == MT - 1))
                        # softmax divide straight into ot (row 0 of po is the
                        # denominator; features sit at partitions 64:128)
                        for hp, po in ((0, poA), (1, poB)):
                            rec = sp.tile([1, 1024], f32, name="rec", tag="rec")
                            nc.vector.reciprocal_approx_fast(
                                out=rec[:], in_=po[0:1, :])
                            recb = dp.tile([128, 1024], f32,
                                           name="recb", tag="recb")
                            nc.gpsimd.partition_broadcast(recb[:], rec[:])
                            nc.vector.tensor_tensor(
                                out=ot_sb[pair][hp * HD:hp * HD + HD,
                                                i0:i0 + 1024],
                                in0=po[HD:128, :], in1=recb[HD:128, :],
                                op=mybir.AluOpType.mult)

                # ---- output projection ----
                for ih in range(2):
                    i0 = ih * 1024
                    for do in range(DT):
                        pso = ps_tile(TAGS[do % 2])
                        for n in range(2):
                            for t in range(FT):
                                nc.tensor.matmul(
                                    pso[:, n * 512:(n + 1) * 512],
                                    lhsT=wo_sb[t][:, do * 128:(do + 1) * 128],
                                    rhs=ot_sb[t][:, i0 + n * 512:
                                                 i0 + (n + 1) * 512],
                                    start=(t == 0), stop=(t == FT - 1))
                        stg = osp.tile([128, 1024], bf16, name="stg", tag="stg")
                        if do % 2 == 0:
                            nc.vector.tensor_scalar_add(
                                stg[:], pso[:], bot[:, do:do + 1])
                        else:
                            nc.scalar.add(stg[:], pso[:], bot[:, do:do + 1])
                        nc.sync.dma_start(
                            out=out_d[do * 128:(do + 1) * 128, i0:i0 + 512],
                            in_=stg[:, 0:512])
                        nc.sync.dma_start(
                            out=out_d[do * 128:(do + 1) * 128,
                                      i0 + 512:i0 + 1024],
                            in_=stg[:, 512:1024])

    nc.compile()
    return nc


def kernel(query, key, value, src_mask, Wq, bq, Wk, bk, Wv, bv, Wo, bo, nhead):
    global LAST_EXEC_NS, LAST_RESULTS
    import ml_dtypes
    from concourse.bass_utils import run_bass_kernel_spmd

    assert int(nhead) == H
    bf16 = ml_dtypes.bfloat16
    query = np.asarray(query, dtype=np.float32)
    key = np.asarray(key, dtype=np.float32)
    value = np.asarray(value, dtype=np.float32)
    src_mask = np.asarray(src_mask)
    Wq, bq = np.asarray(Wq, np.float32), np.asarray(bq, np.float32)
    Wk, bk = np.asarray(Wk, np.float32), np.asarray(bk, np.float32)
    Wv, bv = np.asarray(Wv, np.float32), np.asarray(bv, np.float32)
    Wo, bo = np.asarray(Wo, np.float32), np.asarray(bo, np.float32)

    # host-side key/value compaction
    idxs = [np.nonzero(~src_mask[b])[0] for b in range(B)]
    Ms = [len(ix) for ix in idxs]
    MT = max(2, (max(Ms) + 127) // 128)
    MP = MT * 128

    if ("nc", MT) not in _STATE:
        _STATE[("nc", MT)] = _build(MT)
    nc = _STATE[("nc", MT)]

    xqT, xkT, xvT, maskf = [], [], [], []
    for b in range(B):
        xqT.append(np.ascontiguousarray(query[b].T).astype(bf16))
        kc = np.zeros((MP, D), np.float32)
        kc[:Ms[b]] = key[b][idxs[b]]
        vc = np.zeros((MP, D), np.float32)
        vc[:Ms[b]] = value[b][idxs[b]]
        xkT.append(np.ascontiguousarray(kc.T).astype(bf16))
        xvT.append(np.ascontiguousarray(
            vc.T.reshape(KT, 128, MT, 128).transpose(2, 1, 0, 3)
            .reshape(MT, 128, D)).astype(bf16))
        mk = np.where(np.arange(MP) < Ms[b], np.float32(0), NEG)
        maskf.append(np.ascontiguousarray(mk.reshape(MT, 128).T))

    wqT, wkT, wvT, woT, bqs, bks, bvs = [], [], [], [], [], [], []
    for g in range(NCORES // B):
        gs, ge = g * F, (g + 1) * F
        wqT.append(np.ascontiguousarray(Wq[gs:ge, :].T).astype(bf16))
        wkT.append(np.ascontiguousarray(Wk[gs:ge, :].T).astype(bf16))
        wvT.append(np.ascontiguousarray(Wv[gs:ge, :].T).astype(bf16))
        woT.append(np.ascontiguousarray(Wo[:, gs:ge].T).astype(bf16))
        bqs.append(np.ascontiguousarray(bq[gs:ge].reshape(FT, 128).T))
        bks.append(np.ascontiguousarray(bk[gs:ge].reshape(FT, 128).T))
        bvs.append(np.ascontiguousarray(
            np.tile(bv[gs:ge][None, :], (128, 1)).astype(np.float32)))
    bo2 = np.ascontiguousarray(bo.reshape(DT, 128).T)
    bo_zero = np.zeros_like(bo2)

    in_maps = []
    for c in range(NCORES):
        b, g = c // (NCORES // B), c % (NCORES // B)
        in_maps.append({
            "xqT": xqT[b], "xkT": xkT[b], "xv3": xvT[b],
            "wqT": wqT[g], "wkT": wkT[g], "wvT": wvT[g], "woT": woT[g],
            "bq2": bqs[g], "bk2": bks[g], "bvb": bvs[g],
            "bo2": bo2 if g == 0 else bo_zero,
            "mask2": maskf[b],
        })

    kwargs = {}
    if TRACE:
        kwargs = dict(trace=True)
    res = run_bass_kernel_spmd(nc, in_maps, core_ids=list(range(NCORES)),
                               **kwargs)
    LAST_EXEC_NS = res.exec_time_ns
    LAST_RESULTS = res

    out = np.empty((B, S, D), dtype=np.float32)
    for b in range(B):
        acc = res.results[b * (NCORES // B)]["outT"].astype(np.float32)
        for g in range(1, NCORES // B):
            acc = acc + res.results[b * (NCORES // B) + g]["outT"]
        out[b] = acc.T
    return out


# revision 3
# speedup vs baseline: 1.3290x; 1.0295x over previous
"""Multihead attention (B=2, S=2048, D=1024, H=16) on 8 TRN2 NeuronCores.

Sharding: core c -> batch b = c//4, head-group g = c%4 (4 heads, 256
features). Each core computes q/k/v projections for its 256 features,
attention for its 4 heads, and a row-parallel partial of the output
projection. Host sums the 4 partials per batch and transposes back.

Key compaction: masked keys contribute exactly 0 (exp underflow), so the
host gathers only unmasked keys/values (M_b ~ 1024 of 2048) padded to MT
tiles of 128; pad keys get the -9e9 bias. kproj/vproj/scores/exp/AV all
shrink by ~MT/16.

The TRN2 PE clock ramps with sustained use (0.65/1.2/2.4 GHz; ~3us of
continuous execution to reach max), so the schedule keeps every engine
streaming: attention is split into segments s = (i-half, head-pair); each
segment's scores+exp phase (A_s, ScalarE-paced, PSUM tags SA/SB ring) is
overlapped with the previous segment's AV phase (B_{s-1}, no ScalarE
dependency, 1-bank po tags, n-chunk-serial so two heads fit 2 banks).
vproj fills the PE during A_0; the output projection for i-half 0 fills
the B_3 window. Per window step the AVs are emitted before the next
scores so the PE never idles waiting on exp.

exp is exp(scale*x + maskbias) fused on ScalarE. AV uses the ones-column
trick: va block per head = [ones | zeros(63) | v(64)], so AV row 0 is the
softmax denominator; the divide is DVE reciprocal -> GpSimd partition
broadcast -> DVE multiply written straight into the ot tiles.
"""

import math

import numpy as np

B, S, D, H = 2, 2048, 1024, 16
NCORES = 8
GH = 4                  # heads per core
HD = D // H             # 64
F = GH * HD             # 256 local features
SCALE = 1.0 / math.sqrt(HD)
NEG = np.float32(-9e9)

KT = D // 128           # 8 contraction tiles (projections)
FT = F // 128           # 2 local-feature tiles
DT = D // 128           # 8 output-feature tiles

TRACE = False
LAST_EXEC_NS = None
LAST_RESULTS = None

_STATE = {}


def _build(MT):
    import concourse.bacc as bacc
    import concourse.mybir as mybir
    from concourse.tile import TileContext

    f32 = mybir.dt.float32
    bf16 = mybir.dt.bfloat16
    Exp = mybir.ActivationFunctionType.Exp
    MP = MT * 128

    nc = bacc.Bacc("TRN2", target_bir_lowering=False, debug=False,
                   num_devices=NCORES)

    xq_d = nc.declare_dram_parameter("xqT", [D, S], bf16, isOutput=False)
    xk_d = nc.declare_dram_parameter("xkT", [D, MP], bf16, isOutput=False)
    xv_d = nc.declare_dram_parameter("xv3", [MT, 128, D], bf16, isOutput=False)
    wq_d = nc.declare_dram_parameter("wqT", [D, F], bf16, isOutput=False)
    wk_d = nc.declare_dram_parameter("wkT", [D, F], bf16, isOutput=False)
    wv_d = nc.declare_dram_parameter("wvT", [D, F], bf16, isOutput=False)
    wo_d = nc.declare_dram_parameter("woT", [F, D], bf16, isOutput=False)
    bq_d = nc.declare_dram_parameter("bq2", [128, FT], f32, isOutput=False)
    bk_d = nc.declare_dram_parameter("bk2", [128, FT], f32, isOutput=False)
    bv_d = nc.declare_dram_parameter("bvb", [128, F], f32, isOutput=False)
    bo_d = nc.declare_dram_parameter("bo2", [128, DT], f32, isOutput=False)
    mk_d = nc.declare_dram_parameter("mask2", [128, MT], f32, isOutput=False)
    out_d = nc.declare_dram_parameter("outT", [D, S], bf16, isOutput=True)

    kchunks = []
    c0 = 0
    while c0 < MP:
        w = min(512, MP - c0)
        kchunks.append((c0, w))
        c0 += w

    with TileContext(nc) as tc:
        with tc.tile_pool(name="persist", bufs=1) as pp, \
             tc.tile_pool(name="expp", bufs=38) as ep, \
             tc.tile_pool(name="divp", bufs=2) as dp, \
             tc.tile_pool(name="ostage", bufs=4) as osp, \
             tc.tile_pool(name="small", bufs=2) as sp, \
             tc.tile_pool(name="xkp", bufs=8) as xkp, \
             tc.tile_pool(name="xvp", bufs=4) as xvp, \
             tc.tile_pool(name="xqp", bufs=4) as xqp:

            def ptile(shape, dtype, name):
                return pp.tile(shape, dtype, name=name, tag=name)

            # ---- persistent SBUF tensors ----
            wq_sb = [ptile([128, F], bf16, f"wq{k}") for k in range(KT)]
            wk_sb = [ptile([128, F], bf16, f"wk{k}") for k in range(KT)]
            wv_sb = [ptile([128, F], bf16, f"wv{k}") for k in range(KT)]
            wo_sb = [ptile([128, D], bf16, f"wo{t}") for t in range(FT)]
            bqt = ptile([128, FT], f32, "bqt")
            bkt = ptile([128, FT], f32, "bkt")
            bot = ptile([128, DT], f32, "bot")
            mkt = ptile([128, MT], f32, "mkt")
            bvb = ptile([128, F], f32, "bvb")
            qT_sb = [ptile([128, S], bf16, f"qT{t}") for t in range(FT)]
            kT_sb = [ptile([128, MP], bf16, f"kT{t}") for t in range(FT)]
            va_sb = [ptile([128, GH * 128], bf16, f"va{j}") for j in range(MT)]
            ot_sb = [ptile([128, S], bf16, f"ot{t}") for t in range(FT)]

            for j in range(MT):
                nc.vector.memset(va_sb[j][:], 0.0)
                for h in range(GH):
                    nc.vector.memset(va_sb[j][:, h * 128:h * 128 + 1], 1.0)

            # DMAs in consumption order: k inputs, q inputs, v, wo.
            nc.sync.dma_start(out=mkt[:], in_=mk_d[:])
            nc.sync.dma_start(out=bkt[:], in_=bk_d[:])
            nc.sync.dma_start(out=bqt[:], in_=bq_d[:])
            nc.sync.dma_start(out=bvb[:], in_=bv_d[:])
            nc.sync.dma_start(out=bot[:], in_=bo_d[:])

            xk_sb, xv_sb, xq_sb = [], [], []
            for k in range(KT):
                nc.sync.dma_start(out=wk_sb[k][:],
                                  in_=wk_d[k * 128:(k + 1) * 128, :])
                xt = xkp.tile([128, MP], bf16, name=f"xk{k}", tag="xk")
                nc.sync.dma_start(out=xt[:], in_=xk_d[k * 128:(k + 1) * 128, :])
                xk_sb.append(xt)
            for k in range(KT):
                nc.sync.dma_start(out=wq_sb[k][:],
                                  in_=wq_d[k * 128:(k + 1) * 128, :])
                xt = xqp.tile([128, S], bf16, name=f"xq{k}", tag="xq")
                nc.sync.dma_start(out=xt[:], in_=xq_d[k * 128:(k + 1) * 128, :])
                xq_sb.append(xt)
            for k in range(KT):
                nc.sync.dma_start(out=wv_sb[k][:],
                                  in_=wv_d[k * 128:(k + 1) * 128, :])
            for st in range(MT):
                xt = xvp.tile([128, D], bf16, name=f"xv{st}", tag="xv")
                nc.sync.dma_start(out=xt[:], in_=xv_d[st])
                xv_sb.append(xt)
            for t in range(FT):
                nc.sync.dma_start(out=wo_sb[t][:],
                                  in_=wo_d[t * 128:(t + 1) * 128, :])

            with tc.tile_pool(name="psB", bufs=1, space="PSUM") as psB:

                def ps2(tag):  # 2-bank [128,1024] tile
                    return psB.tile([128, 1024], mybir.dt.float32,
                                    name=tag, tag=tag)

                def ps1(tag):  # 1-bank [128,512] tile
                    return psB.tile([128, 512], mybir.dt.float32,
                                    name=tag, tag=tag)

                # ---- kproj: groups (t, chunk) on SA, SB, T4, T5 ----
                kg = [(t, c0, w) for t in range(FT) for (c0, w) in kchunks]
                accs = []
                ng2 = (len(kg) + 1) // 2 - 1  # groups packed into SA/SB pairs
                tile = None
                for gi, (t, c0, w) in enumerate(kg):
                    if gi < 4:
                        if gi % 2 == 0:
                            tile = ps2("SA" if gi == 0 else "SB")
                        accs.append(tile[:, (gi % 2) * 512:(gi % 2) * 512 + w])
                    else:
                        accs.append(ps1("T4" if gi % 2 == 0 else "T5")[:, 0:w])
                for k in range(KT):
                    for gi, (t, c0, w) in enumerate(kg):
                        nc.tensor.matmul(
                            accs[gi],
                            lhsT=wk_sb[k][:, t * 128:(t + 1) * 128],
                            rhs=xk_sb[k][:, c0:c0 + w],
                            start=(k == 0), stop=(k == KT - 1))
                for gi, (t, c0, w) in enumerate(kg):
                    nc.vector.tensor_scalar_add(
                        kT_sb[t][:, c0:c0 + w], accs[gi], bkt[:, t:t + 1])

                # ---- qproj: 8 groups (t, ch) on SA+SB halves and T4..T7 ----
                qg = [(t, ch) for t in range(FT) for ch in range(4)]
                qaccs = []
                for gi, (t, ch) in enumerate(qg):
                    if gi < 4:
                        if gi % 2 == 0:
                            tile = ps2("SA" if gi == 0 else "SB")
                        qaccs.append(tile[:, (gi % 2) * 512:(gi % 2) * 512 + 512])
                    else:
                        qaccs.append(ps1(f"T{gi}")[:, :])
                for k in range(KT):
                    for gi, (t, ch) in enumerate(qg):
                        nc.tensor.matmul(
                            qaccs[gi],
                            lhsT=wq_sb[k][:, t * 128:(t + 1) * 128],
                            rhs=xq_sb[k][:, ch * 512:(ch + 1) * 512],
                            start=(k == 0), stop=(k == KT - 1))
                for gi, (t, ch) in enumerate(qg):
                    nc.vector.tensor_scalar_add(
                        qT_sb[t][:, ch * 512:(ch + 1) * 512],
                        qaccs[gi], bqt[:, t:t + 1])

                # ---- attention: segments s = (half, pair) ----
                # window w runs segment s's A-phase (scores+exp) against
                # segment s-1's B-phase (AV + divides). A_0 window also runs
                # vproj; the B_3 window also runs out_proj for i-half 0.
                SEGS = [(h, p) for h in range(2) for p in range(FT)]
                e_tiles = {}   # (seg, head01, j) -> exp tile

                def emit_scores(s, j):
                    half, pair = SEGS[s]
                    i0 = half * 1024
                    sa = ps2("SA")
                    sb = ps2("SB")
                    for n in range(2):
                        for off, stile in ((0, sa), (HD, sb)):
                            nc.tensor.matmul(
                                stile[:, n * 512:(n + 1) * 512],
                                lhsT=kT_sb[pair][off:off + HD,
                                                 j * 128:(j + 1) * 128],
                                rhs=qT_sb[pair][off:off + HD,
                                                i0 + n * 512:i0 + (n + 1) * 512],
                                start=True, stop=True)
                    for hp, stile in ((0, sa), (1, sb)):
                        e = ep.tile([128, 1024], bf16, name="e", tag="e")
                        nc.scalar.activation(e[:], stile[:], Exp,
                                             bias=mkt[:, j:j + 1], scale=SCALE)
                        e_tiles[(s, hp, j)] = e

                bstate = {}

                def emit_av(s, w):
                    # B-phase substep w of segment s: n = w//MT, j = w%MT
                    half, pair = SEGS[s]
                    n, j = divmod(w, MT)
                    if j == 0:
                        bstate[(s, n, 0)] = ps1("T4")
                        bstate[(s, n, 1)] = ps1("T5")
                    for hp in range(2):
                        po = bstate[(s, n, hp)]
                        h = 2 * pair + hp
                        nc.tensor.matmul(
                            po[:],
                            lhsT=va_sb[j][:, h * 128:(h + 1) * 128],
                            rhs=e_tiles[(s, hp, j)][:, n * 512:(n + 1) * 512],
                            start=(j == 0), stop=(j == MT - 1))
                    if j == MT - 1:
                        i0 = half * 1024 + n * 512
                        for hp in range(2):
                            po = bstate.pop((s, n, hp))
                            rec = sp.tile([1, 512], f32, name="rec", tag="rec")
                            nc.vector.reciprocal_approx_fast(
                                out=rec[:], in_=po[0:1, :])
                            recb = dp.tile([128, 512], f32,
                                           name="recb", tag="recb")
                            nc.gpsimd.partition_broadcast(recb[:], rec[:])
                            nc.vector.tensor_tensor(
                                out=ot_sb[pair][hp * HD:hp * HD + HD,
                                                i0:i0 + 512],
                                in0=po[HD:128, :], in1=recb[HD:128, :],
                                op=mybir.AluOpType.mult)

                def emit_vproj(st):
                    pv = ps1("T6" if st % 2 == 0 else "T7")
                    for k in range(KT):
                        nc.tensor.matmul(
                            pv[:, 0:F],
                            lhsT=xv_sb[st][:, k * 128:(k + 1) * 128],
                            rhs=wv_sb[k][:], start=(k == 0), stop=(k == KT - 1))
                    for h in range(GH):
                        nc.vector.tensor_tensor(
                            out=va_sb[st][:, h * 128 + HD:(h + 1) * 128],
                            in0=pv[:, h * HD:(h + 1) * HD],
                            in1=bvb[:, h * HD:(h + 1) * HD],
                            op=mybir.AluOpType.add)

                def emit_outproj(ih, do, n):
                    i0 = ih * 1024 + n * 512
                    pso = ps1("T6" if (do * 2 + n) % 2 == 0 else "T7")
                    for t in range(FT):
                        nc.tensor.matmul(
                            pso[:],
                            lhsT=wo_sb[t][:, do * 128:(do + 1) * 128],
                            rhs=ot_sb[t][:, i0:i0 + 512],
                            start=(t == 0), stop=(t == FT - 1))
                    stg = osp.tile([128, 512], bf16, name="stg", tag="stg")
                    if do % 2 == 0:
                        nc.vector.tensor_scalar_add(
                            stg[:], pso[:], bot[:, do:do + 1])
                    else:
                        nc.scalar.add(stg[:], pso[:], bot[:, do:do + 1])
                    nc.sync.dma_start(
                        out=out_d[do * 128:(do + 1) * 128, i0:i0 + 512],
                        in_=stg[:])

                NW = 2 * MT  # window steps per segment

                # A_0 window: scores for seg 0 + vproj fill
                for w in range(NW):
                    if w % 2 == 0:
                        emit_scores(0, w // 2)
                    if w < MT:
                        emit_vproj(w)

                # [A_s+1 || B_s] windows
                for s in range(len(SEGS) - 1):
                    for w in range(NW):
                        emit_av(s, w)
                        if w % 2 == 0:
                            emit_scores(s + 1, w // 2)

                # B_3 window + out_proj for i-half 0
                ojobs = [(0, do, n) for do in range(DT) for n in range(2)]
                oi = 0
                for w in range(NW):
                    emit_av(len(SEGS) - 1, w)
                    if w % 2 == 0 and oi < len(ojobs):
                        emit_outproj(*ojobs[oi])
                        oi += 1
                while oi < len(ojobs):
                    emit_outproj(*ojobs[oi])
                    oi += 1

                # out_proj for i-half 1
                for do in range(DT):
                    for n in range(2):
                        emit_outproj(1, do, n)

    nc.compile()
    return nc


def kernel(query, key, value, src_mask, Wq, bq, Wk, bk, Wv, bv, Wo, bo, nhead):
    global LAST_EXEC_NS, LAST_RESULTS
    import ml_dtypes
    from concourse.bass_utils import run_bass_kernel_spmd

    assert int(nhead) == H
    bf16 = ml_dtypes.bfloat16
    query = np.asarray(query, dtype=np.float32)
    key = np.asarray(key, dtype=np.float32)
    value = np.asarray(value, dtype=np.float32)
    src_mask = np.asarray(src_mask)
    Wq, bq = np.asarray(Wq, np.float32), np.asarray(bq, np.float32)
    Wk, bk = np.asarray(Wk, np.float32), np.asarray(bk, np.float32)
    Wv, bv = np.asarray(Wv, np.float32), np.asarray(bv, np.float32)
    Wo, bo = np.asarray(Wo, np.float32), np.asarray(bo, np.float32)

    # host-side key/value compaction
    idxs = [np.nonzero(~src_mask[b])[0] for b in range(B)]
    Ms = [len(ix) for ix in idxs]
    MT = max(2, (max(Ms) + 127) // 128)
    MP = MT * 128

    if ("nc", MT) not in _STATE:
        _STATE[("nc", MT)] = _build(MT)
    nc = _STATE[("nc", MT)]

    xqT, xkT, xvT, maskf = [], [], [], []
    for b in range(B):
        xqT.append(np.ascontiguousarray(query[b].T).astype(bf16))
        kc = np.zeros((MP, D), np.float32)
        kc[:Ms[b]] = key[b][idxs[b]]
        vc = np.zeros((MP, D), np.float32)
        vc[:Ms[b]] = value[b][idxs[b]]
        xkT.append(np.ascontiguousarray(kc.T).astype(bf16))
        xvT.append(np.ascontiguousarray(
            vc.T.reshape(KT, 128, MT, 128).transpose(2, 1, 0, 3)
            .reshape(MT, 128, D)).astype(bf16))
        mk = np.where(np.arange(MP) < Ms[b], np.float32(0), NEG)
        maskf.append(np.ascontiguousarray(mk.reshape(MT, 128).T))

    wqT, wkT, wvT, woT, bqs, bks, bvs = [], [], [], [], [], [], []
    for g in range(NCORES // B):
        gs, ge = g * F, (g + 1) * F
        wqT.append(np.ascontiguousarray(Wq[gs:ge, :].T).astype(bf16))
        wkT.append(np.ascontiguousarray(Wk[gs:ge, :].T).astype(bf16))
        wvT.append(np.ascontiguousarray(Wv[gs:ge, :].T).astype(bf16))
        woT.append(np.ascontiguousarray(Wo[:, gs:ge].T).astype(bf16))
        bqs.append(np.ascontiguousarray(bq[gs:ge].reshape(FT, 128).T))
        bks.append(np.ascontiguousarray(bk[gs:ge].reshape(FT, 128).T))
        bvs.append(np.ascontiguousarray(
            np.tile(bv[gs:ge][None, :], (128, 1)).astype(np.float32)))
    bo2 = np.ascontiguousarray(bo.reshape(DT, 128).T)
    bo_zero = np.zeros_like(bo2)

    in_maps = []
    for c in range(NCORES):
        b, g = c // (NCORES // B), c % (NCORES // B)
        in_maps.append({
            "xqT": xqT[b], "xkT": xkT[b], "xv3": xvT[b],
            "wqT": wqT[g], "wkT": wkT[g], "wvT": wvT[g], "woT": woT[g],
            "bq2": bqs[g], "bk2": bks[g], "bvb": bvs[g],
            "bo2": bo2 if g == 0 else bo_zero,
            "mask2": maskf[b],
        })

    kwargs = {}
    if TRACE:
        kwargs = dict(trace=True)
    res = run_bass_kernel_spmd(nc, in_maps, core_ids=list(range(NCORES)),
                               **kwargs)
    LAST_EXEC_NS = res.exec_time_ns
    LAST_RESULTS = res

    out = np.empty((B, S, D), dtype=np.float32)
    for b in range(B):
        acc = res.results[b * (NCORES // B)]["outT"].astype(np.float32)
        for g in range(1, NCORES // B):
            acc = acc + res.results[b * (NCORES // B) + g]["outT"]
        out[b] = acc.T
    return out


# revision 12
# speedup vs baseline: 1.5061x; 1.1333x over previous
"""Multihead attention (B=2, S=2048, D=1024, H=16) on 8 TRN2 NeuronCores.

Sharding: core c -> batch b = c//4, head-group g = c%4 (4 heads, 256
features). Each core computes q/k/v projections for its 256 features,
attention for its 4 heads, and a row-parallel partial of the output
projection. Host sums the 4 partials per batch and transposes back.

Key compaction: masked keys contribute exactly 0 (exp underflow), so the
host gathers only unmasked keys/values (M_b ~ 1024 of 2048) padded to MT
tiles of 128; pad keys get the -9e9 bias.

Two hardware behaviors shape the schedule:
 - The PE clock ramps with sustained use (~0.65 -> 2.4 GHz over ~6us of
   continuous execution) and sags on idle gaps, so every phase keeps the
   PE stream dense and handoffs avoid gating the PE on VectorE.
 - 64-row matmuls placed on PE row-tiles T0 (SBUF partitions 0:64) and T8
   (64:128) execute concurrently when alternated (measured 2.8x), but a
   mode switch (128-row matmul) drains the array. So the whole attention
   phase is 64-contraction: scores per head land on T0/T8 by head parity,
   and the AV contraction is split into lo/hi key halves accumulated
   separately (combined by one DVE add before the softmax divide).

Attention runs in segments s = (i-half, head-pair): scores+exp of segment
s (ScalarE-paced, tags SA/SB) overlap the AV phase of segment s-1 (po on
T4..T7, n-chunk serial). vproj fills the PE during the first window; the
output projection fills the last. exp(scale*x + maskbias) is fused on
ScalarE; the divide is DVE reciprocal -> GpSimd partition broadcast ->
DVE multiply straight into the ot tiles.
"""

import math

import numpy as np

B, S, D, H = 2, 2048, 1024, 16
NCORES = 8
GH = 4                  # heads per core
HD = D // H             # 64
F = GH * HD             # 256 local features
SCALE = 1.0 / math.sqrt(HD)
NEG = np.float32(-9e9)

KT = D // 128           # 8 contraction tiles (projections)
FT = F // 128           # 2 local-feature tiles
DT = D // 128           # 8 output-feature tiles

TRACE = False
LAST_EXEC_NS = None
LAST_RESULTS = None

_STATE = {}


def _build(MT):
    import concourse.bacc as bacc
    import concourse.mybir as mybir
    from concourse.tile import TileContext

    f32 = mybir.dt.float32
    bf16 = mybir.dt.bfloat16
    Exp = mybir.ActivationFunctionType.Exp
    MP = MT * 128

    nc = bacc.Bacc("TRN2", target_bir_lowering=False, debug=False,
                   num_devices=NCORES)

    xq_d = nc.declare_dram_parameter("xqT", [D, S], bf16, isOutput=False)
    xk_d = nc.declare_dram_parameter("xkT", [D, MP], bf16, isOutput=False)
    xv_d = nc.declare_dram_parameter("xv3", [MT, 128, D], bf16, isOutput=False)
    wq_d = nc.declare_dram_parameter("wqT", [D, F], bf16, isOutput=False)
    wk_d = nc.declare_dram_parameter("wkT", [D, F], bf16, isOutput=False)
    wv_d = nc.declare_dram_parameter("wvT", [D, F], bf16, isOutput=False)
    wo_d = nc.declare_dram_parameter("woT", [F, D], bf16, isOutput=False)
    bq_d = nc.declare_dram_parameter("bq2", [128, FT], f32, isOutput=False)
    bk_d = nc.declare_dram_parameter("bk2", [128, FT], f32, isOutput=False)
    bv_d = nc.declare_dram_parameter("bvb", [128, F], f32, isOutput=False)
    bo_d = nc.declare_dram_parameter("bo2", [128, DT], f32, isOutput=False)
    mk_d = nc.declare_dram_parameter("mask2", [128, MT], f32, isOutput=False)
    out_d = nc.declare_dram_parameter("outT", [D, S], bf16, isOutput=True)

    kchunks = []
    c0 = 0
    while c0 < MP:
        w = min(512, MP - c0)
        kchunks.append((c0, w))
        c0 += w

    with TileContext(nc) as tc:
        with tc.tile_pool(name="persist", bufs=1) as pp, \
             tc.tile_pool(name="expp", bufs=32) as ep, \
             tc.tile_pool(name="divp", bufs=2) as dp, \
             tc.tile_pool(name="cmbp", bufs=2) as cp, \
             tc.tile_pool(name="ostage", bufs=4) as osp, \
             tc.tile_pool(name="small", bufs=2) as sp, \
             tc.tile_pool(name="xkp", bufs=8) as xkp, \
             tc.tile_pool(name="xvp", bufs=4) as xvp, \
             tc.tile_pool(name="xqp", bufs=8) as xqp:

            def ptile(shape, dtype, name):
                return pp.tile(shape, dtype, name=name, tag=name)

            # ---- persistent SBUF tensors ----
            wq_sb = [ptile([128, F], bf16, f"wq{k}") for k in range(KT)]
            wk_sb = [ptile([128, F], bf16, f"wk{k}") for k in range(KT)]
            wv_sb = [ptile([128, F], bf16, f"wv{k}") for k in range(KT)]
            wo_sb = [ptile([128, D], bf16, f"wo{t}") for t in range(FT)]
            bqt = ptile([128, FT], f32, "bqt")
            bkt = ptile([128, FT], f32, "bkt")
            bot = ptile([128, DT], f32, "bot")
            mkt = ptile([128, MT], f32, "mkt")
            bvb = ptile([128, F], f32, "bvb")
            qT_sb = [ptile([128, S], bf16, f"qT{t}") for t in range(FT)]
            kT_sb = [ptile([128, MP], bf16, f"kT{t}") for t in range(FT)]
            va_sb = [ptile([128, GH * 128], bf16, f"va{j}") for j in range(MT)]
            # ot split per i-half so out_proj for half 0 carries no (false)
            # dependency on half 1's divides
            ot_sb = [[ptile([128, 1024], bf16, f"ot{t}h{hf}") for hf in range(2)]
                     for t in range(FT)]

            for j in range(MT):
                nc.vector.memset(va_sb[j][:], 0.0)
                for h in range(GH):
                    nc.vector.memset(va_sb[j][:, h * 128:h * 128 + 1], 1.0)

            # DMAs in consumption order; big tiles split per 64 partitions
            # so the first tiles land fast across parallel queues.
            nc.sync.dma_start(out=mkt[:], in_=mk_d[:])
            nc.sync.dma_start(out=bkt[:], in_=bk_d[:])
            nc.sync.dma_start(out=bqt[:], in_=bq_d[:])
            nc.sync.dma_start(out=bvb[:], in_=bv_d[:])
            nc.sync.dma_start(out=bot[:], in_=bo_d[:])

            def dma_split(dst, src, parts=2):
                step = 128 // parts
                for i in range(parts):
                    nc.sync.dma_start(out=dst[i * step:(i + 1) * step, :],
                                      in_=src[i * step:(i + 1) * step, :])

            xk_sb, xv_sb, xq_sb = [], [], []
            for k in range(KT):
                nc.sync.dma_start(out=wk_sb[k][:],
                                  in_=wk_d[k * 128:(k + 1) * 128, :])
                xt = xkp.tile([128, MP], bf16, name=f"xk{k}", tag="xk")
                dma_split(xt, xk_d[k * 128:(k + 1) * 128, :])
                xk_sb.append(xt)
            for k in range(KT):
                nc.sync.dma_start(out=wq_sb[k][:],
                                  in_=wq_d[k * 128:(k + 1) * 128, :])
                xt = xqp.tile([128, S], bf16, name=f"xq{k}", tag="xq")
                dma_split(xt, xq_d[k * 128:(k + 1) * 128, :])
                xq_sb.append(xt)
            for k in range(KT):
                nc.sync.dma_start(out=wv_sb[k][:],
                                  in_=wv_d[k * 128:(k + 1) * 128, :])
            for st in range(MT):
                xt = xvp.tile([128, D], bf16, name=f"xv{st}", tag="xv")
                nc.sync.dma_start(out=xt[:], in_=xv_d[st])
                xv_sb.append(xt)
            for t in range(FT):
                nc.sync.dma_start(out=wo_sb[t][:],
                                  in_=wo_d[t * 128:(t + 1) * 128, :])

            with tc.tile_pool(name="psB", bufs=1, space="PSUM") as psB:

                def ps2(tag):  # 2-bank [128,1024] tile
                    return psB.tile([128, 1024], mybir.dt.float32,
                                    name=tag, tag=tag)

                def ps1(tag):  # 1-bank [128,512] tile
                    return psB.tile([128, 512], mybir.dt.float32,
                                    name=tag, tag=tag)

                # ---- kproj: groups (t, chunk) on SA, SB, T4, T5 ----
                kg = [(t, c0, w) for t in range(FT) for (c0, w) in kchunks]
                accs = []
                tile = None
                for gi, (t, c0, w) in enumerate(kg):
                    if gi < 4:
                        if gi % 2 == 0:
                            tile = ps2("SA" if gi == 0 else "SB")
                        accs.append(tile[:, (gi % 2) * 512:(gi % 2) * 512 + w])
                    else:
                        accs.append(ps1("T4" if gi % 2 == 0 else "T5")[:, 0:w])
                for k in range(KT):
                    for gi, (t, c0, w) in enumerate(kg):
                        nc.tensor.matmul(
                            accs[gi],
                            lhsT=wk_sb[k][:, t * 128:(t + 1) * 128],
                            rhs=xk_sb[k][:, c0:c0 + w],
                            start=(k == 0), stop=(k == KT - 1))
                for gi, (t, c0, w) in enumerate(kg):
                    nc.vector.tensor_scalar_add(
                        kT_sb[t][:, c0:c0 + w], accs[gi], bkt[:, t:t + 1])

                # ---- qproj: pass 1 on fresh T6/T7 (the groups attention
                # needs first), then pass 2 on SA/SB/T4/T5 ----
                qg1 = [(0, 0), (0, 1)]
                qg2 = [(0, 2), (0, 3), (1, 0), (1, 1), (1, 2), (1, 3)]

                def q_mm(acc, k, t, ch, first, last):
                    nc.tensor.matmul(
                        acc, lhsT=wq_sb[k][:, t * 128:(t + 1) * 128],
                        rhs=xq_sb[k][:, ch * 512:(ch + 1) * 512],
                        start=first, stop=last)

                qaccs1 = [ps1("T6")[:, :], ps1("T7")[:, :]]
                for k in range(KT):
                    for gi, (t, ch) in enumerate(qg1):
                        q_mm(qaccs1[gi], k, t, ch, k == 0, k == KT - 1)
                for gi, (t, ch) in enumerate(qg1):
                    nc.vector.tensor_scalar_add(
                        qT_sb[t][:, ch * 512:(ch + 1) * 512],
                        qaccs1[gi], bqt[:, t:t + 1])
                qaccs2 = []
                for gi in range(len(qg2)):
                    if gi < 4:
                        if gi % 2 == 0:
                            tile = ps2("SA" if gi == 0 else "SB")
                        qaccs2.append(tile[:, (gi % 2) * 512:(gi % 2) * 512 + 512])
                    else:
                        qaccs2.append(ps1("T4" if gi % 2 == 0 else "T5")[:, :])
                for k in range(KT):
                    for gi, (t, ch) in enumerate(qg2):
                        q_mm(qaccs2[gi], k, t, ch, k == 0, k == KT - 1)
                for gi, (t, ch) in enumerate(qg2):
                    nc.vector.tensor_scalar_add(
                        qT_sb[t][:, ch * 512:(ch + 1) * 512],
                        qaccs2[gi], bqt[:, t:t + 1])

                # ---- attention: segments s = (half, pair) ----
                SEGS = [(h, p) for h in range(2) for p in range(FT)]
                e_tiles = {}

                def emit_scores(s, j):
                    half, pair = SEGS[s]
                    i0 = half * 1024
                    sa = ps2("SA")
                    sb = ps2("SB")
                    for n in range(2):
                        for off, stile in ((0, sa), (HD, sb)):
                            nc.tensor.matmul(
                                stile[:, n * 512:(n + 1) * 512],
                                lhsT=kT_sb[pair][off:off + HD,
                                                 j * 128:(j + 1) * 128],
                                rhs=qT_sb[pair][off:off + HD,
                                                i0 + n * 512:i0 + (n + 1) * 512],
                                start=True, stop=True)
                    for hp, stile in ((0, sa), (1, sb)):
                        e = ep.tile([128, 1024], bf16, name="e", tag="e")
                        nc.scalar.activation(e[:], stile[:], Exp,
                                             bias=mkt[:, j:j + 1], scale=SCALE)
                        e_tiles[(s, hp, j)] = e

                bstate = {}
                PO_TAGS = {(0, 0): "T4", (0, 1): "T5", (1, 0): "T6", (1, 1): "T7"}

                def emit_av(s, w):
                    # B-phase substep w: n = w//MT, j = w%MT; AV contraction
                    # split into lo (T0) / hi (T8) key halves per head.
                    half, pair = SEGS[s]
                    n, j = divmod(w, MT)
                    if j == 0:
                        for hp in range(2):
                            for lh in range(2):
                                bstate[(s, n, hp, lh)] = ps1(PO_TAGS[(hp, lh)])
                    for hp in range(2):
                        h = 2 * pair + hp
                        for lh in range(2):
                            po = bstate[(s, n, hp, lh)]
                            b0 = lh * HD
                            nc.tensor.matmul(
                                po[:],
                                lhsT=va_sb[j][b0:b0 + HD,
                                              h * 128:(h + 1) * 128],
                                rhs=e_tiles[(s, hp, j)][b0:b0 + HD,
                                                        n * 512:(n + 1) * 512],
                                start=(j == 0), stop=(j == MT - 1))
                    if j == MT - 1:
                        i0 = half * 1024 + n * 512
                        cmbs, recs = [], []
                        for hp in range(2):
                            lo = bstate.pop((s, n, hp, 0))
                            hi = bstate.pop((s, n, hp, 1))
                            # DVE cannot read two PSUM operands in one op:
                            # stage lo in SBUF, then add hi (PSUM) to it.
                            los = cp.tile([128, 512], f32, name="los",
                                          tag="los")
                            nc.vector.tensor_copy(los[:], lo[:])
                            cmb = cp.tile([128, 512], f32, name="cmb",
                                          tag="cmb")
                            nc.vector.tensor_tensor(
                                out=cmb[:], in0=hi[:], in1=los[:],
                                op=mybir.AluOpType.add)
                            rec = sp.tile([1, 512], f32, name="rec", tag="rec")
                            nc.vector.reciprocal_approx_fast(
                                out=rec[:], in_=cmb[0:1, :])
                            recb = dp.tile([128, 512], f32,
                                           name="recb", tag="recb")
                            nc.gpsimd.partition_broadcast(recb[:], rec[:])
                            cmbs.append(cmb)
                            recs.append(recb)
                        for hp in range(2):
                            nc.vector.tensor_tensor(
                                out=ot_sb[pair][half][hp * HD:hp * HD + HD,
                                                      n * 512:(n + 1) * 512],
                                in0=cmbs[hp][HD:128, :],
                                in1=recs[hp][HD:128, :],
                                op=mybir.AluOpType.mult)

                def emit_vproj(st):
                    # 64-contraction lo/hi halves on T0/T8 so the A_0 window
                    # stays in one PE tiling mode; combined during bias add.
                    pv_lo = ps1("T6")
                    pv_hi = ps1("T7")
                    for k in range(KT):
                        for b0, pv in ((0, pv_lo), (HD, pv_hi)):
                            nc.tensor.matmul(
                                pv[:, 0:F],
                                lhsT=xv_sb[st][b0:b0 + HD,
                                               k * 128:(k + 1) * 128],
                                rhs=wv_sb[k][b0:b0 + HD, :],
                                start=(k == 0), stop=(k == KT - 1))
                    lvs = cp.tile([128, F], f32, name="lvs", tag="lvs")
                    nc.vector.tensor_copy(lvs[:], pv_lo[:, 0:F])
                    cv = cp.tile([128, F], f32, name="cmbv", tag="cmbv")
                    nc.vector.tensor_tensor(
                        out=cv[:], in0=pv_hi[:, 0:F], in1=lvs[:],
                        op=mybir.AluOpType.add)
                    for h in range(GH):
                        nc.vector.tensor_tensor(
                            out=va_sb[st][:, h * 128 + HD:(h + 1) * 128],
                            in0=cv[:, h * HD:(h + 1) * HD],
                            in1=bvb[:, h * HD:(h + 1) * HD],
                            op=mybir.AluOpType.add)

                def emit_outproj(ih, do):
                    # one pull covers both 512-wide i-chunks of this do-tile
                    i0 = ih * 1024
                    pso = ps2("SA" if do % 2 == 0 else "SB")
                    for n in range(2):
                        for t in range(FT):
                            nc.tensor.matmul(
                                pso[:, n * 512:(n + 1) * 512],
                                lhsT=wo_sb[t][:, do * 128:(do + 1) * 128],
                                rhs=ot_sb[t][ih][:, n * 512:(n + 1) * 512],
                                start=(t == 0), stop=(t == FT - 1))
                    stg = osp.tile([128, 1024], bf16, name="stg", tag="stg")
                    if do % 2 == 0:
                        nc.vector.tensor_scalar_add(
                            stg[:], pso[:], bot[:, do:do + 1])
                    else:
                        nc.scalar.add(stg[:], pso[:], bot[:, do:do + 1])
                    for i in range(2):
                        nc.sync.dma_start(
                            out=out_d[do * 128 + i * 64:do * 128 + (i + 1) * 64,
                                      i0:i0 + 1024],
                            in_=stg[i * 64:(i + 1) * 64, :])

                NW = 2 * MT

                # A_0 window: scores for seg 0 + vproj fill on odd steps
                for w in range(NW):
                    if w % 2 == 0:
                        emit_scores(0, w // 2)
                    elif w // 2 < MT:
                        emit_vproj(w // 2)
                for st in range(NW // 2, MT):
                    emit_vproj(st)

                # [A_s+1 || B_s] windows
                for s in range(len(SEGS) - 1):
                    for w in range(NW):
                        emit_av(s, w)
                        if w % 2 == 0:
                            emit_scores(s + 1, w // 2)

                # B_3 window + out_proj for i-half 0
                oi = 0
                for w in range(NW):
                    emit_av(len(SEGS) - 1, w)
                    if w % 2 == 0 and oi < DT:
                        emit_outproj(0, oi)
                        oi += 1
                while oi < DT:
                    emit_outproj(0, oi)
                    oi += 1

                # out_proj for i-half 1
                for do in range(DT):
                    emit_outproj(1, do)

    nc.compile()
    return nc


def kernel(query, key, value, src_mask, Wq, bq, Wk, bk, Wv, bv, Wo, bo, nhead):
    global LAST_EXEC_NS, LAST_RESULTS
    import ml_dtypes
    from concourse.bass_utils import run_bass_kernel_spmd

    assert int(nhead) == H
    bf16 = ml_dtypes.bfloat16
    query = np.asarray(query, dtype=np.float32)
    key = np.asarray(key, dtype=np.float32)
    value = np.asarray(value, dtype=np.float32)
    src_mask = np.asarray(src_mask)
    Wq, bq = np.asarray(Wq, np.float32), np.asarray(bq, np.float32)
    Wk, bk = np.asarray(Wk, np.float32), np.asarray(bk, np.float32)
    Wv, bv = np.asarray(Wv, np.float32), np.asarray(bv, np.float32)
    Wo, bo = np.asarray(Wo, np.float32), np.asarray(bo, np.float32)

    # host-side key/value compaction
    idxs = [np.nonzero(~src_mask[b])[0] for b in range(B)]
    Ms = [len(ix) for ix in idxs]
    MT = max(2, (max(Ms) + 127) // 128)
    MP = MT * 128

    if ("nc", MT) not in _STATE:
        _STATE[("nc", MT)] = _build(MT)
    nc = _STATE[("nc", MT)]

    xqT, xkT, xvT, maskf = [], [], [], []
    for b in range(B):
        xqT.append(np.ascontiguousarray(query[b].T).astype(bf16))
        kc = np.zeros((MP, D), np.float32)
        kc[:Ms[b]] = key[b][idxs[b]]
        vc = np.zeros((MP, D), np.float32)
        vc[:Ms[b]] = value[b][idxs[b]]
        xkT.append(np.ascontiguousarray(kc.T).astype(bf16))
        xvT.append(np.ascontiguousarray(
            vc.T.reshape(KT, 128, MT, 128).transpose(2, 1, 0, 3)
            .reshape(MT, 128, D)).astype(bf16))
        mk = np.where(np.arange(MP) < Ms[b], np.float32(0), NEG)
        maskf.append(np.ascontiguousarray(mk.reshape(MT, 128).T))

    wqT, wkT, wvT, woT, bqs, bks, bvs = [], [], [], [], [], [], []
    for g in range(NCORES // B):
        gs, ge = g * F, (g + 1) * F
        wqT.append(np.ascontiguousarray(Wq[gs:ge, :].T).astype(bf16))
        wkT.append(np.ascontiguousarray(Wk[gs:ge, :].T).astype(bf16))
        wvT.append(np.ascontiguousarray(Wv[gs:ge, :].T).astype(bf16))
        woT.append(np.ascontiguousarray(Wo[:, gs:ge].T).astype(bf16))
        bqs.append(np.ascontiguousarray(bq[gs:ge].reshape(FT, 128).T))
        bks.append(np.ascontiguousarray(bk[gs:ge].reshape(FT, 128).T))
        bvs.append(np.ascontiguousarray(
            np.tile(bv[gs:ge][None, :], (128, 1)).astype(np.float32)))
    bo2 = np.ascontiguousarray(bo.reshape(DT, 128).T)
    bo_zero = np.zeros_like(bo2)

    in_maps = []
    for c in range(NCORES):
        b, g = c // (NCORES // B), c % (NCORES // B)
        in_maps.append({
            "xqT": xqT[b], "xkT": xkT[b], "xv3": xvT[b],
            "wqT": wqT[g], "wkT": wkT[g], "wvT": wvT[g], "woT": woT[g],
            "bq2": bqs[g], "bk2": bks[g], "bvb": bvs[g],
            "bo2": bo2 if g == 0 else bo_zero,
            "mask2": maskf[b],
        })

    kwargs = {}
    if TRACE:
        kwargs = dict(trace=True)
    res = run_bass_kernel_spmd(nc, in_maps, core_ids=list(range(NCORES)),
                               **kwargs)
    LAST_EXEC_NS = res.exec_time_ns
    LAST_RESULTS = res

    out = np.empty((B, S, D), dtype=np.float32)
    for b in range(B):
        acc = res.results[b * (NCORES // B)]["outT"].astype(np.float32)
        for g in range(1, NCORES // B):
            acc = acc + res.results[b * (NCORES // B) + g]["outT"]
        out[b] = acc.T
    return out
